# revision 21
# baseline (speedup 1.0000x reference)
"""Trainium2 Bass kernel for multi-head attention with RoPE (nn_Attention).

Reference computation (B=1, N=2048, D=1024, 16 heads, hd=64):
    q = x @ wq.T; k = x @ wk.T; v = x @ wv.T      (reshaped to heads)
    q, k = rope(q), rope(k)
    out = softmax(q k^T / sqrt(hd)) v              (non-causal, full)
    return (out reshaped) @ wp.T

Sharding: tensor-parallel over heads — each of the 8 cores owns 2 heads for
QKV projection + SDPA, then an AllToAll redistributes the attention output
so each core computes the final projection for its 256 sequence rows with
the full wp. Matmuls run in float32r (full-rate, ~1.7e-4 rel err).

Self-contained: only imports numpy + the concourse stack available in the
execution environment. kernel(**inputs) takes the full unsharded inputs and
returns the full output.
"""
import numpy as np

DIM = 1024
NHEADS = 16
HD = 64
SEQ = 2048
NCORES = 8
ROPE_BASE = 10000.0
HPC = NHEADS // NCORES      # heads per core = 2
CH = HPC * HD               # channels per core = 128
QCH = 512                   # q-chunk (free dim of S/P tiles)
NQC = SEQ // QCH            # 4
NKT = SEQ // 128            # 16 k-tiles
DCH = DIM // 128            # 8 contraction chunks

_CACHE = {}


def _rope_tables():
    inv = 1.0 / (ROPE_BASE ** (np.arange(0, HD, 2, dtype=np.float64) / HD))
    t = np.arange(SEQ, dtype=np.float64)
    freqs = np.outer(t, inv)                      # [SEQ, 32]
    emb = np.concatenate([freqs, freqs], 1)       # [SEQ, 64]
    cosT = np.cos(emb).T                          # [64, SEQ]
    sinT = np.sin(emb).T
    sig = (np.arange(HD) + 32) % HD
    sT = sinT[sig]                                # shifted sin
    cos2 = np.concatenate([cosT, cosT], 0)        # [128, SEQ] (2 heads)
    s2 = np.concatenate([sT, sT], 0)
    return cos2, s2


def _r2t():
    # rotate-half matrix R (per head), block-diagonal over the 2 heads; we
    # pass R2.T as the stationary matmul operand.
    R = np.zeros((HD, HD), np.float64)
    for j in range(32):
        R[j, j + 32] = -1.0
        R[j + 32, j] = 1.0
    R2 = np.zeros((CH, CH), np.float64)
    R2[0:HD, 0:HD] = R
    R2[HD:CH, HD:CH] = R
    return np.ascontiguousarray(R2.T).astype(np.float32)


def _build(nrep=1, n_cores=NCORES, with_c=True, parts="ab"):
    import concourse.mybir as mybir
    import concourse.tile as tile
    from concourse import bacc
    from concourse.masks import make_identity

    F32 = mybir.dt.float32
    F32R = mybir.dt.float32r
    EXP = mybir.ActivationFunctionType.Exp

    nc = bacc.Bacc("TRN2", target_bir_lowering=False, debug=False,
                   num_devices=n_cores)

    xt_ext = nc.dram_tensor("xt", [DIM, SEQ], F32, kind="ExternalInput")
    wq_ext = nc.dram_tensor("wq_t", [DIM, CH], F32, kind="ExternalInput")
    wk_ext = nc.dram_tensor("wk_t", [DIM, CH], F32, kind="ExternalInput")
    wv_ext = nc.dram_tensor("wv_t", [DIM, CH], F32, kind="ExternalInput")
    wp_ext = nc.dram_tensor("wp_t", [DIM, DIM], F32, kind="ExternalInput")
    ck_ext = nc.dram_tensor("cos_k", [CH, SEQ], F32, kind="ExternalInput")
    sk_ext = nc.dram_tensor("sin_k", [CH, SEQ], F32, kind="ExternalInput")
    r2t_ext = nc.dram_tensor("r2t", [CH, CH], F32, kind="ExternalInput")
    out_ext = nc.dram_tensor("out", [SEQ // NCORES, DIM], F32,
                             kind="ExternalOutput")
    a2a_in = nc.dram_tensor("a2a_in", [NCORES, CH, SEQ // NCORES], F32)
    a2a_out = nc.dram_tensor("a2a_out", [NCORES, CH, SEQ // NCORES], F32)

    with tile.TileContext(nc) as tc:

        def stage_ab(Qp, Kp, Vsb, onescol, parts="ab"):
            # One unified scope for projections + attention so the Tile
            # scheduler can overlap attention chunks with later Q chunks.
            # PSUM budget (8 banks): big (2-bank slots x2) + small (1-bank
            # x2) + oaug (1-bank x2).
            with (
                tc.tile_pool(name="stA", bufs=1) as A_sb,
                tc.tile_pool(name="stA2", bufs=2) as A_db,
                tc.tile_pool(name="psBig", bufs=2, space="PSUM") as psBig,
                tc.tile_pool(name="psSm", bufs=2, space="PSUM") as psSm,
                tc.tile_pool(name="psO", bufs=2, space="PSUM") as psO,
                tc.tile_pool(name="stB", bufs=2) as B_db,
                tc.tile_pool(name="stBs", bufs=3) as B_sm,
            ):
                if "a" not in parts:
                    _attention(Qp, Kp, Vsb, onescol, A_db, B_db, B_sm,
                               psBig, psSm, psO, None, None, None, False)
                    return
                aux1 = A_sb.tile([128, HD], F32, tag="aux1")
                nc.vector.memset(aux1[:], 1.0)
                nc.vector.tensor_copy(onescol[:], aux1[:])
                nc.vector.tensor_copy(
                    Vsb[:, :, :, HD],
                    aux1[:, 0:NKT * HPC].rearrange("p (k h) -> p k h", h=HPC))

                # ---- stage A inputs. float32r-typed DMAs are ~75x
                # slower than f32 on this platform, so everything lands in
                # f32 staging tiles and is round-copied to f32r on idle
                # engines (ACT for xt during the DMA lead-in, DVE for the
                # small weights).
                wq = A_sb.tile([128, DCH, CH], F32R, tag="wq")
                wk = A_sb.tile([128, DCH, CH], F32R, tag="wk")
                wv = A_sb.tile([128, DCH, CH], F32R, tag="wv")
                xt = A_sb.tile([128, DCH, SEQ], F32R, tag="xt")
                xt_r = xt_ext.rearrange("(c p) n -> p c n", p=128)
                r2t = A_sb.tile([CH, CH], F32R, tag="r2t")
                ck = A_sb.tile([CH, SEQ], F32, tag="ck")
                sk = A_sb.tile([CH, SEQ], F32, tag="sk")
                wkf = A_db.tile([128, DCH, CH], F32, tag="wf")
                nc.sync.dma_start(
                    out=wkf[:], in_=wk_ext.rearrange("(c p) j -> p c j", p=128))
                nc.vector.tensor_copy(wk[:], wkf[:])
                r2tf = A_db.tile([CH, CH], F32, tag="r2tf")
                nc.sync.dma_start(out=r2tf[:], in_=r2t_ext[:])
                nc.vector.tensor_copy(r2t[:], r2tf[:])
                wvf = A_db.tile([128, DCH, CH], F32, tag="wf")
                nc.sync.dma_start(
                    out=wvf[:], in_=wv_ext.rearrange("(c p) j -> p c j", p=128))
                nc.vector.tensor_copy(wv[:], wvf[:])
                for d in range(DCH):
                    xtf = A_db.tile([128, SEQ], F32, tag="xtf")
                    nc.sync.dma_start(out=xtf[:], in_=xt_r[:, d, :])
                    nc.scalar.copy(xt[:, d, :], xtf[:])
                nc.sync.dma_start(out=sk[:], in_=sk_ext[:])
                nc.sync.dma_start(out=ck[:], in_=ck_ext[:])
                wqf = A_db.tile([128, DCH, CH], F32, tag="wf")
                nc.sync.dma_start(
                    out=wqf[:], in_=wq_ext.rearrange("(c p) j -> p c j", p=128))
                nc.vector.tensor_copy(wq[:], wqf[:])
                ident = A_sb.tile([128, 128], F32, tag="ident")
                make_identity(nc, ident[:])
                identr = A_sb.tile([128, 128], F32R, tag="identr")
                nc.vector.tensor_copy(identr[:], ident[:])

                # ---- projections: K first, then V, then Q — attention
                # q-chunks only need Q' chunk-by-chunk, so emitting Q last
                # lets attention overlap the tail of the projections.
                def qk_proj(w_sb, cos_sb, sin_sb, dst, qc):
                    sl = slice(qc * QCH, (qc + 1) * QCH)
                    ps_q = psSm.tile([CH, QCH], F32, tag="sm")
                    for d in range(DCH):
                        nc.tensor.matmul(ps_q[:], w_sb[:, d, :],
                                         xt[:, d, sl],
                                         start=(d == 0), stop=(d == DCH - 1))
                    qs = A_db.tile([CH, QCH], F32R, tag="qs")
                    nc.vector.tensor_mul(qs[:], ps_q[:], sin_sb[:, sl])
                    qct = A_db.tile([CH, QCH], F32R, tag="qct")
                    nc.vector.tensor_mul(qct[:], ps_q[:], cos_sb[:, sl])
                    nc.tensor.matmul(ps_q[:], r2t[:], qs[:],
                                     start=True, stop=True)
                    nc.vector.tensor_add(dst[:, sl], qct[:], ps_q[:])

                # K and V projections, d-outer: all 8 chunk-accumulators
                # live at once (4 K halves in the two 2-bank "big" slots,
                # 4 V chunks in the four 1-bank slots), so the first xt
                # d-chunk to arrive immediately feeds 8 matmuls.
                kacc0 = psBig.tile([128, HPC, QCH], F32, tag="big")
                kacc1 = psBig.tile([128, HPC, QCH], F32, tag="big")
                vacc0 = psSm.tile([CH, QCH], F32, tag="sm")
                vacc1 = psSm.tile([CH, QCH], F32, tag="sm")
                vacc2 = psO.tile([CH, QCH], F32, tag="oaug")
                vacc3 = psO.tile([CH, QCH], F32, tag="oaug")
                kaccs = [kacc0[:, 0, :], kacc0[:, 1, :],
                         kacc1[:, 0, :], kacc1[:, 1, :]]
                vaccs = [vacc0, vacc1, vacc2, vacc3]
                for d in range(DCH):
                    st, sp = d == 0, d == DCH - 1
                    for c in range(NQC):
                        slc = slice(c * QCH, (c + 1) * QCH)
                        nc.tensor.matmul(kaccs[c], wk[:, d, :], xt[:, d, slc],
                                         start=st, stop=sp)
                        nc.tensor.matmul(vaccs[c][:], wv[:, d, :],
                                         xt[:, d, slc], start=st, stop=sp)

                # RoPE for K: the rot matmul overwrites the K-accumulator
                # bank in place (start=True) after both DVE reads, so no
                # extra PSUM slot is needed.
                for c in range(NQC):
                    slc = slice(c * QCH, (c + 1) * QCH)
                    qs = A_db.tile([CH, QCH], F32R, tag="qs")
                    nc.vector.tensor_mul(qs[:], kaccs[c], sk[:, slc])
                    qct = A_db.tile([CH, QCH], F32R, tag="qct")
                    nc.vector.tensor_mul(qct[:], kaccs[c], ck[:, slc])
                    nc.tensor.matmul(kaccs[c], r2t[:], qs[:],
                                     start=True, stop=True)
                    nc.vector.tensor_add(Kp[:, slc], qct[:], kaccs[c])

                qk_proj(wq, ck, sk, Qp, 0)

                # V: copy out of psum, then PE-transpose into Vsb (emitted
                # after Q0 so the attention-critical path starts sooner)
                for c in range(NQC):
                    vt = A_db.tile([CH, QCH], F32R, tag="vt")
                    nc.scalar.copy(vt[:], vaccs[c][:])
                    for b in range(QCH // 128):
                        kti = c * (QCH // 128) + b
                        ps_t = psSm.tile([128, 128], F32R, tag="sm")
                        nc.tensor.transpose(
                            ps_t[:], vt[:, b * 128:(b + 1) * 128], identr[:])
                        nc.vector.tensor_copy(
                            Vsb[:, kti, :, 0:HD],
                            ps_t[:].rearrange("p (h j) -> p h j", h=HPC))

                if "b" in parts:
                    _attention(Qp, Kp, Vsb, onescol, A_db, B_db, B_sm,
                               psBig, psSm, psO, qk_proj, wq, (ck, sk), True)
                else:
                    for qc in range(1, NQC):
                        qk_proj(wq, ck, sk, Qp, qc)

        def _attention(Qp, Kp, Vsb, onescol, A_db, B_db, B_sm,
                       psBig, psSm, psO, qk_proj, wq, cs, interleave):
                # ---- attention per head pair, interleaved with the
                # projection of the next Q chunk (hides Q under exp) ----
                for qc in range(NQC):
                    if interleave and qc + 1 < NQC:
                        qk_proj(wq, cs[0], cs[1], Qp, qc + 1)
                    sl = slice(qc * QCH, (qc + 1) * QCH)
                    o_ps0 = psO.tile([HD + 1, QCH], F32, tag="oaug")
                    o_ps1 = psO.tile([HD + 1, QCH], F32, tag="oaug")
                    o_ps = [o_ps0, o_ps1]
                    for kt in range(NKT):
                        s_ps = psBig.tile([128, HPC, QCH], F32, tag="big")
                        for h in range(HPC):
                            nc.tensor.matmul(
                                s_ps[:, h, :],
                                Kp[h * HD:(h + 1) * HD,
                                   kt * 128:(kt + 1) * 128],
                                Qp[h * HD:(h + 1) * HD, sl],
                                start=True, stop=True,
                                tile_position=(h * HD, 0))
                        p_sb = B_db.tile([128, HPC, QCH], F32R, tag="p")
                        nc.scalar.activation(out=p_sb[:], in_=s_ps[:], func=EXP)
                        for h in range(HPC):
                            nc.tensor.matmul(
                                o_ps[h][:], Vsb[:, kt, h, :], p_sb[:, h, :],
                                start=(kt == 0), stop=(kt == NKT - 1))
                    for h in range(HPC):
                        rec = B_sm.tile([HD + 1, QCH], F32R, tag="rec")
                        with nc.allow_low_precision(
                                reason="f32r is fp32-width; rounding only"):
                            nc.vector.reciprocal(rec[HD:HD + 1, :],
                                                 o_ps[h][HD:HD + 1, :])
                        rb_ps = psSm.tile([HD, QCH], F32, tag="sm")
                        nc.tensor.matmul(rb_ps[:], onescol[HD:HD + 1, :],
                                         rec[HD:HD + 1, :],
                                         start=True, stop=True,
                                         tile_position=(HD, 0))
                        rb = B_sm.tile([HD, QCH], F32R, tag="rb_sb")
                        nc.vector.tensor_copy(rb[:], rb_ps[:])
                        on = B_db.tile([HD, QCH], F32, tag="on")
                        nc.vector.tensor_mul(on[:], o_ps[h][0:HD, :], rb[:])
                        # scatter the two 256-blocks of this q-chunk into
                        # a2a_in (destination cores 2*qc and 2*qc+1)
                        for half in range(2):
                            r = 2 * qc + half
                            nc.sync.dma_start(
                                out=a2a_in[r, h * HD:(h + 1) * HD, :],
                                in_=on[:, half * 256:(half + 1) * 256])

        def stage_c():
            with (
                tc.tile_pool(name="stC", bufs=1) as C_sb,
                tc.tile_pool(name="stC2", bufs=2) as C_db,
                tc.tile_pool(name="psC", bufs=2, space="PSUM") as psC,
            ):
                wp = C_sb.tile([128, DCH, DIM], F32R, tag="wp")
                wpf = C_sb.tile([128, DCH, DIM], F32, tag="wpf")
                nc.sync.dma_start(
                    out=wpf[:], in_=wp_ext.rearrange("(s p) o -> p s o", p=128))
                nc.gpsimd.tensor_copy(wp[:], wpf[:])
                nc.gpsimd.collective_compute(
                    "AllToAll", mybir.AluOpType.bypass,
                    replica_groups=[list(range(NCORES))],
                    ins=[a2a_in[:]], outs=[a2a_out[:]])
                gaf = C_sb.tile([CH, NCORES, 256], F32, tag="gaf")
                nc.sync.dma_start(out=gaf[:],
                                  in_=a2a_out.rearrange("r p n -> p r n"))
                ga = C_sb.tile([CH, NCORES, 256], F32R, tag="ga")
                nc.vector.tensor_copy(ga[:], gaf[:])
                for nt in range(2):
                    for oc in range(2):
                        pp = psC.tile([128, 512], F32, tag="pp")
                        for src in range(NCORES):
                            nc.tensor.matmul(
                                pp[:], ga[:, src, nt * 128:(nt + 1) * 128],
                                wp[:, src, oc * 512:(oc + 1) * 512],
                                start=(src == 0), stop=(src == NCORES - 1))
                        ob = C_db.tile([128, 512], F32, tag="ob")
                        nc.vector.tensor_copy(ob[:], pp[:])
                        nc.sync.dma_start(
                            out=out_ext[nt * 128:(nt + 1) * 128,
                                        oc * 512:(oc + 1) * 512],
                            in_=ob[:])

        with tc.tile_pool(name="persist", bufs=1) as P1:
            Qp = P1.tile([CH, SEQ], F32R, tag="Qp")
            Kp = P1.tile([CH, SEQ], F32R, tag="Kp")
            Vsb = P1.tile([128, NKT, HPC, HD + 1], F32R, tag="Vsb")
            onescol = P1.tile([128, HD], F32R, tag="onescol")
            if nrep == 1:
                if parts == "b":
                    stage_ab(Qp, Kp, Vsb, onescol, "a")
                stage_ab(Qp, Kp, Vsb, onescol, parts)
                if with_c:
                    stage_c()
            else:
                # timing build: loop stages A+B (a collective inside a For_i
                # desyncs the mesh), run stage C once after the loop.
                if parts == "b":
                    stage_ab(Qp, Kp, Vsb, onescol, "a")
                with tc.For_i(0, nrep, 1) as _i:
                    stage_ab(Qp, Kp, Vsb, onescol, parts)
                if with_c:
                    stage_c()

    nc.compile()
    return nc


def _get_nc(nrep=1, n_cores=NCORES, with_c=True, parts="ab"):
    key = ("nc", nrep, n_cores, with_c, parts)
    if key not in _CACHE:
        _CACHE[key] = _build(nrep, n_cores, with_c, parts)
    return _CACHE[key]


def _prep_in_maps(x, wq, wk, wv, wp):
    x2 = np.ascontiguousarray(np.asarray(x, np.float32).reshape(SEQ, DIM))
    xt = np.ascontiguousarray(x2.T)
    wq = np.asarray(wq, np.float32)
    wk = np.asarray(wk, np.float32)
    wv = np.asarray(wv, np.float32)
    wp = np.asarray(wp, np.float32)
    cos2, s2 = _rope_tables()
    scale = 1.0 / np.sqrt(HD)
    wq = wq * scale
    ck = np.ascontiguousarray(cos2).astype(np.float32)
    sk = np.ascontiguousarray(s2).astype(np.float32)
    r2t = _r2t()
    wpt = np.ascontiguousarray(wp.T)
    maps = []
    for c in range(NCORES):
        ch = slice(c * CH, (c + 1) * CH)
        maps.append({
            "xt": xt,
            "wq_t": np.ascontiguousarray(wq[ch, :].T),
            "wk_t": np.ascontiguousarray(wk[ch, :].T),
            "wv_t": np.ascontiguousarray(wv[ch, :].T),
            "wp_t": wpt,
            "cos_k": ck, "sin_k": sk,
            "r2t": r2t,
        })
    return maps


def kernel(x, wq, wk, wv, wp):
    from concourse.bass_utils import run_bass_kernel_spmd

    nc = _get_nc(1)
    maps = _prep_in_maps(x, wq, wk, wv, wp)
    res = run_bass_kernel_spmd(nc, maps, list(range(NCORES))).results
    out = np.concatenate([res[c]["out"] for c in range(NCORES)], axis=0)
    return out.reshape(1, SEQ, DIM).astype(np.float32)
